# revision 12
# baseline (speedup 1.0000x reference)
"""Trainium2 Bass kernel for nn_BackProjNet (segment_reduce).

out[c, v] = (sum_r x[c, idx[v, r]] * w[v, r]) * SCALE + bias[v]

Strategy (8 NeuronCores, voxel-sharded):
  - Each core owns 8192 voxels (voxel v -> partition p = v//64, layer jj = v%64).
  - Positions (92160) are split into 3 "colors" (sub-tables of <=31744 rows so
    row ids fit int16); a greedy balanced coloring + repair keeps the max
    per-(voxel, color) ref count K small (less padding).
  - Per (voxel, color) refs are padded to K slots (weight 0 on pads); gathers
    run as InstDMAGatherAnt (32B payload, 256B row stride), 12288 indices per
    instruction on 4 SWDGE queues.  Slot streams are color-major so large
    instructions stay within one color's sub-table.
  - DVE multiplies by the weight stream (broadcast over the 8 channels) and
    segment-reduces 8 voxel layers at a time; bias is added once at the end.
"""

import os
import sys

import numpy as np

for _p in ("/opt/trn_rl_repo", "/root/.axon_site/_ro/trn_rl_repo"):
    if _p not in sys.path:
        sys.path.append(_p)

import concourse.bass as bass
import concourse.bacc as bacc
import concourse.mybir as mybir
import concourse.tile as tile
from concourse import ap_utils
from concourse._compat import exact_div
from concourse.bass import round_up_to_multiple
from concourse.bass_interp import get_hw_module

# geometry (must match reference.py)
CHANNEL = 8
NVX, NVY = 256, 256
VIEWS, EXTENT = 180, 2
NDETU = 512
SCALE = (2.0 * np.pi - 0.0) / (2.0 * VIEWS * EXTENT)

NCORES = 8
P = 128
V = VIEWS * NDETU          # 92160 sinogram positions
R = VIEWS * EXTENT         # 360 rays per voxel
NVOX = NVX * NVY           # 65536 voxels
VPC = NVOX // NCORES       # 8192 voxels per core
JPP = VPC // P             # 64 voxels per partition ("layers")
COLORS = 3
CSIZE = 31744              # rows per color sub-table (int16-safe, slack>V/3)
PITCH = 64                 # f32 per table row (256B)
NPI = int(os.environ.get("KNPI", "12288"))  # indices per gather instruction
SPI = NPI // P             # slots per partition per instruction
IDXREP = bool(int(os.environ.get("KIDXREP", "0")))  # replicate idx on host
JG = 8                     # voxel layers per staged tile
NST_J = JPP // JG          # 8 jj-groups
NST = COLORS * NST_J       # 24 staged tiles per core


def _dma_gather_raw(gpsimd, out_ap, in_ap, idxs_ap, num_idxs, elem_size,
                    elem_step, queue_num):
    """bass.dma_gather without the elem_size%256 restriction (the 256B
    constraint is on the row stride, which we satisfy with PITCH=64 f32)."""
    self = gpsimd
    assert idxs_ap.dtype == mybir.dt.int16
    assert in_ap.space == bass.MemorySpace.DRAM
    assert in_ap.dtype == out_ap.dtype
    assert idxs_ap.space == bass.MemorySpace.SBUF
    assert out_ap.space == bass.MemorySpace.SBUF
    assert ap_utils.ap_is_contiguous(out_ap.ap[1:])
    assert ap_utils.ap_is_contiguous(idxs_ap.ap[1:])
    assert in_ap.ap[-1][1] == out_ap.ap[-1][1] == elem_size
    assert out_ap.ap[0][1] * out_ap.ap[1][1] == round_up_to_multiple(num_idxs, 128)
    assert in_ap.ap[0][0] == elem_step
    stride_bytes = elem_step * mybir.dt.size(in_ap.dtype)
    stride_bytes_256 = exact_div(stride_bytes, 256)
    _in_ap = self.lower_ap_dma(in_ap, for_custom_bir_dma=True)
    _idxs_ap = self.lower_ap(idxs_ap)
    _out_ap = self.lower_ap(out_ap)
    return self.add_instruction(
        mybir.InstDMAGatherAnt(
            name=self.bass.get_next_instruction_name(),
            ins=[*_in_ap, _idxs_ap, self.lower_val_access(self.to_reg(num_idxs))],
            outs=[_out_ap],
            transpose=False,
            num_idxs=num_idxs,
            elem_size=elem_size,
            stride_bytes_256=stride_bytes_256,
            gen_mode=0,
            single_packet=True,
            queue_num=queue_num,
            sbuf_tokens_per_rank=0,
            sbuf_free_dim_per_rank=0,
            sbuf_free_dim_pad_per_rank=0,
            sbuf_byte_offset=0,
        )
    )


# ---------------------------------------------------------------- coloring

def _color_positions(idx2, rng):
    """Greedy balanced 3-coloring of the 92160 positions for one core.

    idx2: [VPC, R] position of each ref.  Returns (color [V] int8,
    counts [VPC, COLORS] int32) where counts[v, c] = refs of voxel v with
    color c.  Greedy minimizes a convex load penalty over the voxels that
    reference each position; capacity per color is CSIZE.
    """
    flat = idx2.ravel()
    order = np.argsort(flat, kind="stable")
    vv = (order // R).astype(np.int32)
    pos_sorted = flat[order]
    start = np.searchsorted(pos_sorted, np.arange(V + 1))
    deg = (start[1:] - start[:-1]).astype(np.int64)
    maxdeg = int(deg.max())
    adj = np.full((V, maxdeg), VPC, np.int32)  # sentinel = VPC
    ranks = np.arange(flat.size, dtype=np.int64) - np.repeat(start[:-1], deg)
    adj[pos_sorted, ranks] = vv

    counts = np.zeros((VPC + 1, COLORS), np.int32)
    pen = np.exp((np.arange(512) - 118.0) / 6.0).astype(np.float32)
    color = np.empty(V, np.int8)
    capleft = np.array([CSIZE] * COLORS, np.int64)

    pperm = rng.permutation(V)
    CH = 512
    for i in range(0, V, CH):
        chunk = pperm[i:i + CH]
        a = adj[chunk]                              # [CH, maxdeg]
        s = pen[counts[a, :]].sum(axis=1)           # [CH, COLORS]
        s[:, capleft <= 0] = np.inf
        ch = np.argmin(s, axis=1).astype(np.int8)
        # rough per-chunk capacity guard
        bc = np.bincount(ch, minlength=COLORS)
        for c in range(COLORS):
            if bc[c] > capleft[c]:
                over = np.flatnonzero(ch == c)[capleft[c]:]
                alt = np.argsort(s[over], axis=1)
                for j, o in enumerate(over):
                    for cand in alt[j]:
                        if cand != c and capleft[cand] - np.count_nonzero(
                                ch[over[:j]] == cand) > bc[cand]:
                            ch[o] = cand
                            break
        bc = np.bincount(ch, minlength=COLORS)
        capleft -= bc
        color[chunk] = ch
        fl = a.ravel().astype(np.int64) * COLORS + np.repeat(
            ch.astype(np.int64), maxdeg)
        counts.ravel()[:] += np.bincount(fl, minlength=counts.size).astype(
            np.int32)
        counts[VPC] = 0
    return color, counts[:VPC], adj, capleft


def _repair_coloring(idx2, color, counts, adj, capleft, k_t):
    """Move positions between colors until max per-(voxel,color) count <= k_t.
    Returns achieved K (may exceed k_t if repair stalls)."""
    for _ in range(4):  # escalation rounds
        stalled = False
        for _pass in range(20000):
            viol = np.argwhere(counts > k_t)
            if viol.size == 0:
                return k_t
            v, c = viol[0]
            vpos = idx2[v]
            cand = vpos[color[vpos] == c]
            moved = False
            for q in cand:
                u = adj[q]
                u = u[u < VPC]
                best, best_load = -1, None
                for c2 in range(COLORS):
                    if c2 == c or capleft[c2] <= 0:
                        continue
                    load = counts[u, c2].max()
                    if load <= k_t - 1 and (best_load is None or load < best_load):
                        best, best_load = c2, load
                if best >= 0:
                    color[q] = best
                    counts[u, c] -= 1
                    counts[u, best] += 1
                    capleft[c] += 1
                    capleft[best] -= 1
                    moved = True
                    if counts[v, c] <= k_t:
                        break
            if not moved:
                stalled = True
                break
        if not stalled:
            continue
        k_t += 12
    return max(k_t, int(np.ceil(counts.max() / 12.0)) * 12)


# ---------------------------------------------------------------- packing

def _prep_core(x, w2, idx2, bias_m, color, counts, K):
    """Build one core's device arrays given the coloring and uniform K."""
    c_ref = color[idx2].astype(np.int64)              # [VPC, R]
    rowof = np.empty(V, np.int64)
    tab = np.zeros((COLORS * CSIZE, PITCH), np.float32)
    for c in range(COLORS):
        pc = np.flatnonzero(color == c)
        rowof[pc] = np.arange(pc.size)
        tab[c * CSIZE:c * CSIZE + pc.size, :CHANNEL] = x[:, pc].T
    loc = rowof[idx2]                                 # [VPC, R] < CSIZE

    order = np.argsort(c_ref + np.linspace(0, 0.9, R)[None, :], axis=1,
                       kind="stable")
    cs = np.take_along_axis(c_ref, order, axis=1)
    ls = np.take_along_axis(loc, order, axis=1)
    ws = np.take_along_axis(w2, order, axis=1)
    starts = np.concatenate([np.zeros((VPC, 1), np.int64),
                             np.cumsum(counts, axis=1)[:, :-1].astype(np.int64)],
                            axis=1)
    pos_in_seg = np.arange(R)[None, :] - np.take_along_axis(starts, cs, axis=1)
    assert int(pos_in_seg.max()) < K
    vv = np.repeat(np.arange(VPC), R)
    idx16 = np.zeros((VPC, COLORS, K), np.int16)
    wpack = np.zeros((VPC, COLORS, K), np.float32)
    idx16[vv, cs.ravel(), pos_in_seg.ravel()] = ls.ravel().astype(np.int16)
    wpack[vv, cs.ravel(), pos_in_seg.ravel()] = ws.ravel() * np.float32(SCALE)

    # stream layout per partition: s = (c*JPP + jj)*K + k ; voxel = p*JPP + jj
    idx_s = idx16.reshape(P, JPP, COLORS, K).transpose(0, 2, 1, 3)  # [p,c,jj,k]
    w_s = wpack.reshape(P, JPP, COLORS, K).transpose(0, 2, 1, 3)
    L = COLORS * JPP * K
    idx_s = np.ascontiguousarray(idx_s.reshape(P, L))
    w_s = np.ascontiguousarray(w_s.reshape(P, L))

    # weights grouped per staged tile: [NST, P, JG*K]
    w_dev = np.ascontiguousarray(
        w_s.reshape(P, NST, JG * K).transpose(1, 0, 2))

    # per-instruction idx lists, wrapped in 16 partitions: [nInst, 16, NPI//16]
    n_inst = L // SPI
    arr = idx_s.reshape(P, n_inst, SPI).transpose(1, 2, 0)   # [n, q, p]
    arr = arr.reshape(n_inst, NPI)                            # i = q*128+p
    idx_dev = np.ascontiguousarray(
        arr.reshape(n_inst, NPI // 16, 16).transpose(0, 2, 1))

    bias_dev = np.ascontiguousarray(
        np.repeat(bias_m.reshape(P, JPP), CHANNEL).reshape(P, JPP * CHANNEL))

    return dict(tab=tab, idx=idx_dev, wts=w_dev, biasx=bias_dev)


# ---------------------------------------------------------------- module

def _build_module(K, debug_sts=None, hw=True):
    n_inst_st = exact_div(JG * K, SPI)
    n_inst = NST * n_inst_st
    nc = bacc.Bacc(
        "TRN2",
        target_bir_lowering=False,
        debug=False,
        num_devices=NCORES,
        dynamic_dma_scratch_size=16384,
        num_swdge_queues=4,
    )
    tab_d = nc.dram_tensor("tab", [COLORS * CSIZE, PITCH], mybir.dt.float32,
                           kind="ExternalInput")
    idx_d = nc.dram_tensor("idx", [n_inst, 16, NPI // 16], mybir.dt.int16,
                           kind="ExternalInput")
    w_d = nc.dram_tensor("wts", [NST, P, JG * K], mybir.dt.float32,
                         kind="ExternalInput")
    b_d = nc.dram_tensor("biasx", [P, JPP * CHANNEL], mybir.dt.float32,
                         kind="ExternalInput")
    out_d = nc.dram_tensor("out", [P, JPP * CHANNEL], mybir.dt.float32,
                           kind="ExternalOutput")

    tab_ap = tab_d.ap()
    idx_ap = idx_d.ap()
    w_ap = w_d.ap()

    with tile.TileContext(nc) as tc:
        with (
            tc.tile_pool(name="const", bufs=1) as cp,
            tc.tile_pool(name="wstream", bufs=2) as wp,
            tc.tile_pool(name="istream", bufs=4) as ip,
            tc.tile_pool(name="gat", bufs=2) as gp,
        ):
            bias_t = cp.tile([P, JPP * CHANNEL], mybir.dt.float32)
            out_t = cp.tile([P, JPP * CHANNEL], mybir.dt.float32)
            accs = [cp.tile([P, JPP * CHANNEL], mybir.dt.float32, name=f"acc{c}")
                    for c in range(COLORS)]
            nc.sync.dma_start(out=bias_t[:], in_=b_d.ap())
            if debug_sts is not None:
                for a in accs:
                    nc.vector.memset(a[:], 0.0)

            qn = 0
            for st in (range(NST) if debug_sts is None else debug_sts):
                c, jjg = st // NST_J, st % NST_J
                g_t = gp.tile([P, JG * K * CHANNEL], mybir.dt.float32, tag="g")
                w_t = wp.tile([P, JG * K], mybir.dt.float32, tag="w")
                nc.sync.dma_start(out=w_t[:], in_=w_ap[st])
                src = tab_ap[c * CSIZE:(c + 1) * CSIZE, :CHANNEL]
                for n_local in range(n_inst_st):
                    n = st * n_inst_st + n_local
                    idx_t = ip.tile([P, NPI // 16], mybir.dt.int16, tag="idx")
                    a = idx_ap[n]
                    bcast = bass.AP(a.tensor, a.offset, [[0, P // 16]] + list(a.ap))
                    nc.sync.dma_start(out=idx_t[:], in_=bcast)
                    out_ap = g_t[:, n_local * SPI * CHANNEL:
                                 (n_local + 1) * SPI * CHANNEL].rearrange(
                        "p (q e) -> p q e", e=CHANNEL)
                    _dma_gather_raw(
                        nc.gpsimd,
                        out_ap=out_ap,
                        in_ap=src,
                        idxs_ap=idx_t[:],
                        num_idxs=NPI,
                        elem_size=CHANNEL,
                        elem_step=PITCH,
                        queue_num=qn % 4,
                    )
                    qn += 1
                g3 = g_t[:].rearrange("p (k e) -> p k e", e=CHANNEL)
                wb = w_t[:].to_broadcast([P, JG * K, CHANNEL])
                nc.vector.tensor_tensor(out=g3, in0=g3, in1=wb,
                                        op=mybir.AluOpType.mult)
                gr = g_t[:].rearrange("p (s k e) -> p s e k", s=JG, e=CHANNEL)
                acc_sl = accs[c][:, jjg * JG * CHANNEL:(jjg + 1) * JG * CHANNEL]
                acc3 = acc_sl.rearrange("p (s e) -> p s e", s=JG)
                nc.vector.tensor_reduce(out=acc3, in_=gr,
                                        axis=mybir.AxisListType.X,
                                        op=mybir.AluOpType.add)
            nc.vector.tensor_tensor(out=out_t[:], in0=accs[0][:], in1=accs[1][:],
                                    op=mybir.AluOpType.add)
            nc.vector.tensor_tensor(out=out_t[:], in0=out_t[:], in1=accs[2][:],
                                    op=mybir.AluOpType.add)
            nc.vector.tensor_tensor(out=out_t[:], in0=out_t[:], in1=bias_t[:],
                                    op=mybir.AluOpType.add)
            nc.sync.dma_start(out=out_d.ap(), in_=out_t[:])

    nc.compile()
    if hw:
        nc.m = get_hw_module(nc.m)
    return nc


class _Runner:
    """Compile once, execute the SPMD module on 8 cores via PJRT."""

    def __init__(self, nc, n_cores):
        import jax
        from jax.sharding import Mesh, PartitionSpec
        from jax.experimental.shard_map import shard_map
        from concourse.bass2jax import (_bass_exec_p, partition_id_tensor,
                                        install_neuronx_cc_hook)

        install_neuronx_cc_hook()
        self.jax = jax
        self.n_cores = n_cores
        in_names, out_names, out_avals = [], [], []
        pname = nc.partition_id_tensor.name if nc.partition_id_tensor else None
        for alloc in nc.m.functions[0].allocations:
            if not isinstance(alloc, mybir.MemoryLocationSet):
                continue
            name = alloc.memorylocations[0].name
            if alloc.kind == "ExternalInput":
                if name != pname:
                    in_names.append(name)
            elif alloc.kind == "ExternalOutput":
                out_names.append(name)
                out_avals.append(jax.core.ShapedArray(
                    tuple(alloc.tensor_shape), mybir.dt.np(alloc.dtype)))
        self.in_names, self.out_names, self.out_avals = in_names, out_names, out_avals
        all_in = list(in_names) + list(out_names) + ([pname] if pname else [])

        def _body(*args):
            operands = list(args)
            if pname is not None:
                operands.append(partition_id_tensor())
            return tuple(_bass_exec_p.bind(
                *operands, out_avals=tuple(out_avals), in_names=tuple(all_in),
                out_names=tuple(out_names), lowering_input_output_aliases=(),
                sim_require_finite=True, sim_require_nnan=True, nc=nc))

        devices = jax.devices()[:n_cores]
        self.mesh = Mesh(np.asarray(devices), ("core",))
        nin = len(in_names) + len(out_names)
        self.fn = jax.jit(
            shard_map(_body, mesh=self.mesh,
                      in_specs=(PartitionSpec("core"),) * nin,
                      out_specs=(PartitionSpec("core"),) * len(out_names),
                      check_rep=False),
            keep_unused=True)
        self._dev_in = None

    def set_inputs(self, in_maps):
        import jax
        from jax.sharding import NamedSharding, PartitionSpec
        sh = NamedSharding(self.mesh, PartitionSpec("core"))
        n = self.n_cores
        cat = [np.concatenate([np.asarray(in_maps[c][nm]) for c in range(n)], axis=0)
               for nm in self.in_names]
        zeros = [np.zeros((n * a.shape[0], *a.shape[1:]), a.dtype)
                 for a in self.out_avals]
        self._dev_in = [jax.device_put(x, sh) for x in cat + zeros]

    def run(self):
        outs = self.fn(*self._dev_in)
        self.jax.block_until_ready(outs)
        return outs

    def outputs_np(self, outs):
        n = self.n_cores
        return [
            {nm: np.asarray(outs[i]).reshape(n, *self.out_avals[i].shape)[c]
             for i, nm in enumerate(self.out_names)}
            for c in range(n)
        ]


_CACHE = {}


def _get_runner(K):
    if K not in _CACHE:
        nc = _build_module(K)
        _CACHE[K] = _Runner(nc, NCORES)
    return _CACHE[K]


def prepare(x, weight, bias, indices):
    """Host-side marshalling: shard + color + build per-core device arrays."""
    x = np.asarray(x, np.float32).reshape(CHANNEL, V)
    weight = np.asarray(weight, np.float32).reshape(NVOX, R)
    bias = np.asarray(bias, np.float32).reshape(NVOX)
    indices = np.asarray(indices).astype(np.int64).reshape(NVOX, R)

    cores = []
    Ks = []
    for m in range(NCORES):
        sl = slice(m * VPC, (m + 1) * VPC)
        idx2 = indices[sl]
        rng = np.random.default_rng(1234 + m)
        color, counts, adj, capleft = _color_positions(idx2, rng)
        k0 = int(np.ceil(max(int(counts.max()), 126) / 12.0)) * 12
        K = _repair_coloring(idx2, color, counts, adj, capleft,
                             max(k0 - 12, 120))
        assert counts.max() <= K
        cores.append((idx2, weight[sl], bias[sl], color, counts))
        Ks.append(K)
    K = max(Ks)

    in_maps = []
    for m in range(NCORES):
        idx2, w2, bias_m, color, counts = cores[m]
        in_maps.append(_prep_core(x, w2, idx2, bias_m, color, counts, K))
    return K, in_maps


def _sim_core(in_map, K):
    """Numpy emulation of the device program for one core (layout check)."""
    tab = in_map["tab"]
    idxd = in_map["idx"]
    w_dev = in_map["wts"]
    n_inst_st = exact_div(JG * K, SPI)
    acc = np.zeros((P, JPP, CHANNEL), np.float32)
    for st in range(NST):
        c, jjg = st // NST_J, st % NST_J
        g = np.empty((P, JG * K, CHANNEL), np.float32)
        for n_local in range(n_inst_st):
            n = st * n_inst_st + n_local
            lst = idxd[n].transpose(1, 0).reshape(NPI).astype(np.int64)
            gath = tab[c * CSIZE + lst, :CHANNEL]       # [NPI, 8]
            g[:, n_local * SPI:(n_local + 1) * SPI] = (
                gath.reshape(SPI, P, CHANNEL).transpose(1, 0, 2))
        g *= w_dev[st][:, :, None]
        red = g.reshape(P, JG, K, CHANNEL).sum(axis=2)
        acc[:, jjg * JG:(jjg + 1) * JG] += red
    acc += in_map["biasx"].reshape(P, JPP, CHANNEL)
    return acc.reshape(P, JPP * CHANNEL)


def kernel(x, weight, bias, indices):
    K, in_maps = prepare(x, weight, bias, indices)
    runner = _get_runner(K)
    runner.set_inputs(in_maps)
    outs = runner.run()
    per_core = runner.outputs_np(outs)
    full = np.empty((1, CHANNEL, NVOX), np.float32)
    for m in range(NCORES):
        o = per_core[m]["out"].reshape(P, JPP, CHANNEL)
        full[0, :, m * VPC:(m + 1) * VPC] = o.transpose(2, 0, 1).reshape(CHANNEL, VPC)
    return full.reshape(1, CHANNEL, NVX, NVY)
